# revision 18
# baseline (speedup 1.0000x reference)
"""Causal3DTransformerBlock on 8 TRN2 NeuronCores.

Sharding: self-attention is head-parallel with ONE head x BOTH batches per
core (core h owns head h).  A single 8-core AllToAll redistributes the
attention outputs to token-parallel (core j: batch j//4, tokens
(j%4)*512 .. +512); every A2A block is real data and the gathered rows are
head-major on every core, so the out-projection (full C contraction) runs
after the exchange with the unpermuted w_attn_out.  Cross-attention and the
SwiGLU FFN are token-parallel (no further collectives).

LN1 is applied host-side (x-hat is the kernel input), so phase A is pure
projection matmuls + RoPE.  Phase A and the causal attention (phase B) are
emitted interleaved per (batch, megagroup) so the softmax Exp overlaps the
projection matmuls.  Softmax ships RAW exp-weighted sums plus the
denominator row through the A2A (o rows + a tiny second den AllToAll) and
normalizes on the receiving core, off the phase-B critical path.
LN2/LN3 rsqrt runs on DVE (quake seed + 1 Newton step) so the Scalar
engine never reloads activation tables between Exp and Ln.
The FFN runs in fp8 (e4m3, weights x64) with DoubleRow matmuls; SiLU on
the ACT engine descales the gate.
Large weights (wqc, wo, wco, wgu, wd) are host-pre-tiled and streamed
through small double-buffered SBUF tiles.
"""

import sys

sys.path.insert(0, "/opt/trn_rl_repo")

import os

import numpy as np
import ml_dtypes

BF16 = ml_dtypes.bfloat16
SKIP_COLLECTIVE = bool(int(os.environ.get("K_SKIP_COLLECTIVE", "0")))

B, S, C, CTX, II, H, DH = 2, 2048, 768, 128, 3072, 8, 96
NCORES = 8
TG = 512         # tokens per core after the exchange (A2A block width)
MG = 1024        # megagroup width for self-attention phases
NMG = S // MG    # 2
NKT = S // 128   # 16 key tiles
KPG = MG // 128  # 8 key tiles per megagroup
NCT = C // 128   # 6 feature tiles
NIT = II // 128  # 24 FFN intermediate tiles
DP = 128         # stored (permuted+padded) Q/K head dim
NQB = S // TG    # 4 q-blocks of 512
EPS = 1e-5
RG8 = [[0, 1, 2, 3, 4, 5, 6, 7]]
MAGIC = 0x5F3759DF
USE_DVE_RSQRT = bool(int(os.environ.get("K_DVE_RSQRT", "1")))

_CACHE = {}


def _build_program(bias_zero):
    import concourse.bass as bass
    import concourse.tile as tile
    from concourse import bacc, mybir
    from concourse.alu_op_type import AluOpType as alu

    f32 = mybir.dt.float32
    bf16 = mybir.dt.bfloat16
    i32 = mybir.dt.int32
    AF = mybir.ActivationFunctionType

    nc = bacc.Bacc("TRN2", debug=False, num_devices=NCORES)

    def din(name, shape, dt=bf16):
        return nc.dram_tensor(name, shape, dt, kind="ExternalInput").ap()

    # x-hat (LN1 applied host-side), pre-tiled [128, (g,c) blocks of MG cols]
    xh = [din(f"xh{b}", [128, NMG * NCT * MG]) for b in range(B)]
    x_own = din("x_own", [C, TG], f32)    # own 512-token slice, fp32
    ctx_bf = din("ctx_bf", [128, NCT * CTX])  # own batch ctx^T, pre-tiled
    cosT = din("cosT", [DP, S])
    sinT = din("sinT", [DP, S])           # sign-folded, partner-swapped (^64)
    tri = din("tri", [128, 128])          # triu {0,1} mask: [k,q] valid q>=k
    ones_in = din("ones_in", [128, 128])
    # merged [wq|wk|wv] pre-tiled: block c at cols c*(DP+DP+DH)
    wqkv = din("wqkv_t", [128, NCT * (2 * DP + DH)])
    wkc = din("wkc", [128, NCT * C])      # pre-tiled, block c at cols c*C
    wvc = din("wvc", [128, NCT * C])
    # pre-tiled [128, blocks]: see _prep_inputs for the layouts
    wo_t = din("wo_t", [128, NCT * NCT * 128])
    wqc_t = din("wqc_t", [128, H * NCT * DH])
    wco_t = din("wco_t", [128, NCT * NCT * 128])
    # FFN weights in fp8 (x64 scaled), DoubleRow-packed; gate/up merged:
    # block it is [wg block (768) | wu block (768)]
    fp8 = mybir.dt.float8e4
    wgu_t = din("wgu_t", [128, NIT * 2 * 3 * 256], fp8)
    wd_t = din("wd_t", [128, NCT * 12 * 256], fp8)
    cqc0 = din("cqc0", [1, C])            # LN2 corr row: -colsum(W')/C
    if not bias_zero:
        cg = din("cg", [1, II])           # b@Wg (bias variant only)
        cu = din("cu", [1, II])

    out_x = nc.dram_tensor("out_x", [C, TG], f32, kind="ExternalOutput").ap()

    with tile.TileContext(nc) as tc:
        with (
            tc.tile_pool(name="const", bufs=1) as cpool,
            tc.tile_pool(name="resid", bufs=1) as rpool,
            tc.tile_pool(name="work", bufs=2) as wpool,
            tc.tile_pool(name="stat", bufs=1) as spool,
            tc.tile_pool(name="dram", bufs=1, space="DRAM") as dpool,
        ):
            # ---- const APs for activation bias ----
            czero = cpool.tile([128, 1], f32, tag="czero", name="czero")
            nc.vector.memset(czero[:], 0.0)
            nc.const_aps.aps[(f32, 0.0)] = czero[:]

            ones_sb = cpool.tile([128, 128], bf16, tag="ones", name="ones")
            tri_sb = cpool.tile([128, 128], bf16, tag="tri", name="tri")

            def mmF(ps, lhsT, rhs, c0, c1, start, stop):
                """matmul on cols [c0:c1) of ps/rhs, split at the PSUM bank
                boundary (512 f32 cols)."""
                pts = sorted({c0, c1} | ({512} if c0 < 512 < c1 else set()))
                for a, b in zip(pts, pts[1:]):
                    nc.tensor.matmul(ps[:, a:b], lhsT, rhs[:, a:b],
                                     start=start, stop=stop)

            def rsqrt_dve(out_bf, v, W, tags=("ry", "ra")):
                """out_bf (bf16 [128,W]) = 1/sqrt(v), v f32 [128,W] SBUF.
                Quake-III seed + one Newton step (max rel err ~0.18%)."""
                y = spool.tile([128, W], f32, tag=tags[0], name="rsq_y")
                a = spool.tile([128, W], f32, tag=tags[1], name="rsq_a")
                vi = v.bitcast(i32)
                yi = y[:].bitcast(i32)
                nc.vector.tensor_scalar(yi, vi, 1, None, alu.arith_shift_right)
                nc.vector.tensor_scalar(yi, yi, -1, MAGIC,
                                        alu.mult, alu.add)
                nc.vector.tensor_tensor(a[:], v, y[:], alu.mult)
                nc.vector.tensor_tensor(a[:], a[:], y[:], alu.mult)
                nc.vector.tensor_scalar(a[:], a[:], -0.5, 1.5,
                                        alu.mult, alu.add)
                nc.vector.tensor_tensor(a[:], a[:], y[:], alu.mult)
                nc.vector.tensor_copy(out_bf, a[:])

            def ln_finish(s1, s2, W, want_mu_bc, b1tag="b1",
                          want_corr=True):
                """From accumulated s1/s2 (PSUM [128,W]) produce b1 (bf16
                rsqrt row-broadcast), corr [2,W], optional mu_bc."""
                t_mu = spool.tile([128, W], f32, tag="t_mu", name="t_mu")
                t_m2 = spool.tile([128, W], f32, tag="t_m2", name="t_m2")
                t_v = spool.tile([128, W], f32, tag="t_mu", name="t_v")
                nc.vector.tensor_scalar(t_mu[:], s1[:], 1.0 / C, None,
                                        alu.mult)
                # t_m2 = mu^2 - eps so that t_v = var + eps
                nc.vector.tensor_tensor(t_m2[:], t_mu[:], t_mu[:], alu.mult)
                nc.vector.tensor_scalar(t_m2[:], t_m2[:], EPS, None,
                                        alu.subtract)
                nc.vector.scalar_tensor_tensor(t_v[:], s2[:], 1.0 / C,
                                               t_m2[:], alu.mult,
                                               alu.subtract)
                b1 = spool.tile([128, W], bf16, tag=b1tag, name="b1", bufs=2)
                if USE_DVE_RSQRT:
                    rsqrt_dve(b1[:], t_v[:], W)
                else:
                    t_ln = spool.tile([128, W], f32, tag="t_m2", name="t_ln")
                    nc.scalar.activation(t_ln[:], t_v[:], AF.Ln)
                    nc.scalar.activation(b1[:], t_ln[:], AF.Exp, scale=-0.5)
                corr = None
                if want_corr:
                    corr = spool.tile([2, W], bf16, tag="corr", name="corr",
                                      bufs=2)
                    mu_bf = spool.tile([1, W], bf16, tag="mu_bf",
                                       name="mu_bf")
                    nc.vector.tensor_scalar(mu_bf[:], s1[0:1, :], 1.0 / C,
                                            None, alu.mult)
                    nc.vector.memset(corr[0:2, :], 1.0)
                    nc.vector.tensor_tensor(corr[0:1, :], mu_bf[:],
                                            b1[0:1, :], alu.mult)
                mu_bc = None
                if want_mu_bc:
                    mu_bc = spool.tile([128, W], bf16, tag="mu_bc",
                                       name="mu_bc")
                    nc.vector.tensor_scalar(mu_bc[:], s1[:], 1.0 / C, None,
                                            alu.mult)
                return b1, corr, mu_bc

            # ---------------- startup DMAs (priority order) ----------------
            WQKV = 2 * DP + DH
            wqkv_sb = cpool.tile([128, NCT * WQKV], bf16, tag="wqkv",
                                 name="wqkv")
            nc.sync.dma_start(wqkv_sb[:, 0:WQKV], wqkv[:, 0:WQKV])
            nc.sync.dma_start(wqkv_sb[:, WQKV:NCT * WQKV],
                              wqkv[:, WQKV:NCT * WQKV])
            wq_sb = [wqkv_sb[:, c * WQKV:c * WQKV + DP] for c in range(NCT)]
            wk_sb = [wqkv_sb[:, c * WQKV + DP:c * WQKV + 2 * DP]
                     for c in range(NCT)]
            wv_sb = [wqkv_sb[:, c * WQKV + 2 * DP:(c + 1) * WQKV]
                     for c in range(NCT)]

            with tc.tile_pool(name="attn", bufs=1) as apool:
                # x-hat tiles: (b,g) double-buffered, per-c-tile DMA pieces
                xhat = [[apool.tile([128, NCT * MG], bf16, tag="xh",
                                    name="xh", bufs=2) for _ in range(NMG)]
                        for _ in range(B)]

                def xh_dma(b, g):
                    for c in range(NCT):
                        nc.sync.dma_start(
                            xhat[b][g][:, c * MG:(c + 1) * MG],
                            xh[b][:, (g * NCT + c) * MG:
                                  (g * NCT + c + 1) * MG])

                xh_dma(0, 0)
                cos_sb = [apool.tile([DP, MG], bf16, tag=f"cos{g}",
                                     name=f"cos{g}") for g in range(NMG)]
                sin_sb = [apool.tile([DP, MG], bf16, tag=f"sin{g}",
                                     name=f"sin{g}") for g in range(NMG)]
                nc.sync.dma_start(cos_sb[0][:], cosT[:, 0:MG])
                nc.sync.dma_start(sin_sb[0][:], sinT[:, 0:MG])
                nc.sync.dma_start(tri_sb[:], tri)
                nc.sync.dma_start(ones_sb[:], ones_in)
                xh_dma(1, 0)
                nc.sync.dma_start(cos_sb[1][:], cosT[:, MG:2 * MG])
                nc.sync.dma_start(sin_sb[1][:], sinT[:, MG:2 * MG])

                q_sb = [[apool.tile([DP, MG], bf16, tag=f"q{bb}_{g}",
                                    name=f"q{bb}_{g}") for g in range(NMG)]
                        for bb in range(B)]
                k_sb = [[apool.tile([DP, MG], bf16, tag=f"k{bb}_{g}",
                                    name=f"k{bb}_{g}") for g in range(NMG)]
                        for bb in range(B)]
                v_sb = [[apool.tile([128, DH + 1], bf16, tag=f"v{bb}_{t}",
                                    name=f"v{bb}_{t}") for t in range(NKT)]
                        for bb in range(B)]

                a2a_in = [dpool.tile([H * DH, 256], bf16,
                                     name=f"a2a_in{i}") for i in range(2)]
                a2a_out = [dpool.tile([H * DH, 256], bf16,
                                      name=f"a2a_out{i}") for i in range(2)]
                bar_in = dpool.tile([8, 8], bf16, name="bar_in")
                bar_out = dpool.tile([8, 8], bf16, name="bar_out")

                # pre-barrier: absorb inter-core startup skew on the CC
                # engine while phase A computes, so the real A2A sees
                # aligned peers
                bar_sb = cpool.tile([8, 8], bf16, tag="bar", name="bar")
                nc.vector.memset(bar_sb[:], 1.0)
                nc.sync.dma_start(bar_in[:], bar_sb[:])
                if not SKIP_COLLECTIVE:
                    nc.gpsimd.collective_compute(
                        "AllToAll", alu.bypass, replica_groups=RG8,
                        ins=[bar_in.opt()], outs=[bar_out.opt()])

                with (
                    tc.tile_pool(name="ps_proj", bufs=3,
                                 space="PSUM") as ps_pr,
                    tc.tile_pool(name="ps_sc", bufs=2, space="PSUM") as ps_sc,
                    tc.tile_pool(name="ps_av", bufs=3, space="PSUM") as ps_av,
                ):
                    def phaseA(bb, g):
                        xa = lambda c: xhat[bb][g][:, c * MG:(c + 1) * MG]
                        # q/k projections + RoPE, 512-col grain
                        for wt, dst in ((wq_sb, q_sb), (wk_sb, k_sb)):
                            for hf in range(2):
                                sl = slice(hf * 512, hf * 512 + 512)
                                ps = ps_pr.tile([DP, 512], f32, tag="proj",
                                                name="ps")
                                for c in range(NCT):
                                    nc.tensor.matmul(ps[:], wt[c][:],
                                                     xa(c)[:, sl],
                                                     start=(c == 0),
                                                     stop=(c == NCT - 1))
                                # RoPE: out[d] = ps[d]*cos + ps[d^64]*sin'
                                t1 = apool.tile([DP, 512], bf16, tag="rope1",
                                                name="t1", bufs=3)
                                t2 = apool.tile([DP, 512], bf16, tag="rope2",
                                                name="t2", bufs=3)
                                nc.vector.tensor_tensor(
                                    t1[:], ps[:], cos_sb[g][:, sl], alu.mult)
                                nc.vector.tensor_tensor(
                                    t2[0:64, :], ps[64:128, :],
                                    sin_sb[g][64:128, sl], alu.mult)
                                nc.vector.tensor_tensor(
                                    t2[64:128, :], ps[0:64, :],
                                    sin_sb[g][0:64, sl], alu.mult)
                                nc.vector.tensor_tensor(dst[bb][g][:, sl],
                                                        t1[:], t2[:],
                                                        alu.add)
                        for tt in range(KPG):
                            kt = g * KPG + tt
                            ps = ps_pr.tile([128, DH], f32, tag="proj",
                                            name="ps")
                            for c in range(NCT):
                                nc.tensor.matmul(
                                    ps[:], xa(c)[:, tt * 128:(tt + 1) * 128],
                                    wv_sb[c][:], start=(c == 0),
                                    stop=(c == NCT - 1))
                            nc.scalar.copy(v_sb[bb][kt][:, 0:DH], ps[:])
                            nc.vector.memset(v_sb[bb][kt][:, DH:DH + 1], 1.0)

                    def phaseB(bb, qb):
                        """Causal attention for q-block qb (512 tokens)."""
                        g = qb // 2
                        o_ps = ps_av.tile([DH + 1, 512], f32, tag="av",
                                          name="o_ps")
                        nkt = 4 * qb + 4
                        for kt in range(nkt):
                            i = kt - 4 * qb   # >=0 on the diagonal strip
                            q0 = 0 if i < 0 else i * 128
                            s_ps = ps_sc.tile([128, 512], f32, tag="scores",
                                              name="s_ps")
                            nc.tensor.matmul(
                                s_ps[:, q0:512],
                                k_sb[bb][kt // KPG][
                                    :, (kt % KPG) * 128:(kt % KPG + 1) * 128],
                                q_sb[bb][g][:, (qb % 2) * 512 + q0:
                                            (qb % 2) * 512 + 512],
                                start=True, stop=True)
                            pt = apool.tile([128, 512], bf16, tag="ptB",
                                            name="pt", bufs=3)
                            nc.scalar.activation(pt[:, q0:512],
                                                 s_ps[:, q0:512], AF.Exp)
                            if i >= 0:
                                nc.vector.tensor_tensor(
                                    pt[:, q0:q0 + 128], pt[:, q0:q0 + 128],
                                    tri_sb[:], alu.mult)
                            nc.tensor.matmul(o_ps[:, q0:512], v_sb[bb][kt][:],
                                             pt[:, q0:512], start=(kt == 0),
                                             stop=(kt == nkt - 1))
                        # normalize off the critical path (next qb's
                        # matmuls proceed on other PSUM bufs meanwhile)
                        with nc.allow_low_precision(
                                reason="softmax reciprocal bf16"):
                            o_raw = apool.tile([DH + 1, 512], bf16,
                                               tag="oraw", name="o_raw",
                                               bufs=2)
                            nc.scalar.copy(o_raw[:], o_ps[:])
                            den = spool.tile([1, 512], f32, tag="denB",
                                             name="den", bufs=2)
                            nc.vector.tensor_copy(den[:], o_ps[DH:DH + 1, :])
                            rcp = spool.tile([1, 512], f32, tag="rcpB",
                                             name="rcp", bufs=2)
                            nc.vector.reciprocal_approx_fast(rcp[:], den[:])
                            rcpb = spool.tile([1, 512], bf16, tag="rcpbB",
                                              name="rcpb", bufs=2)
                            nc.vector.tensor_copy(rcpb[:], rcp[:])
                            b_ps = ps_av.tile([DH, 512], f32, tag="av",
                                              name="b_ps")
                            nc.tensor.matmul(b_ps[:], ones_sb[0:1, 0:DH],
                                             rcpb[:], start=True, stop=True)
                            o_n = apool.tile([DH, 512], bf16, tag="on",
                                             name="o_n", bufs=2)
                            nc.vector.tensor_tensor(o_n[:], b_ps[:],
                                                    o_raw[0:DH, :], alu.mult)
                        tgt = a2a_in[qb // 2]
                        for half in range(2):
                            j = bb * 4 + (qb % 2) * 2 + half
                            nc.gpsimd.dma_start(
                                tgt[j * DH:(j + 1) * DH, :],
                                o_n[:, half * 256:(half + 1) * 256])

                    x2 = [rpool.tile([128, TG], f32, tag=f"x2_{c}",
                                     name=f"x2_{c}") for c in range(NCT)]
                    x3 = [rpool.tile([128, TG], f32, tag=f"x3_{c}",
                                     name=f"x3_{c}") for c in range(NCT)]
                    xo_sb = [rpool.tile([128, TG], f32, tag=f"xo{c}",
                                        name=f"xo{c}") for c in range(NCT)]

                    for g in range(NMG):
                        for bb in range(B):
                            if (g, bb) == (0, 1):
                                xh_dma(0, 1)
                            if (g, bb) == (1, 0):
                                xh_dma(1, 1)
                            phaseA(bb, g)
                            phaseB(bb, 2 * g)
                            phaseB(bb, 2 * g + 1)
                        if SKIP_COLLECTIVE:
                            nc.sync.dma_start(a2a_out[g][:], a2a_in[g][:])
                        else:
                            nc.gpsimd.collective_compute(
                                "AllToAll", alu.bypass, replica_groups=RG8,
                                ins=[a2a_in[g].opt()],
                                outs=[a2a_out[g].opt()])
                        if g == 0:
                            # residuals + D-side weights: issue while the
                            # sync queue is unblocked (o_n writes live on
                            # the gpsimd queue)
                            for c in range(NCT):
                                nc.sync.dma_start(
                                    xo_sb[c][:],
                                    x_own[c * 128:(c + 1) * 128, :])

            with (
                tc.tile_pool(name="cross", bufs=1) as xpool,
                tc.tile_pool(name="wstr", bufs=1) as wstr,
                tc.tile_pool(name="ps_projD", bufs=2, space="PSUM") as ps_pD,
                tc.tile_pool(name="ps_attn2", bufs=1, space="PSUM") as ps_at2,
            ):
                wkc_sb = xpool.tile([128, NCT * C], bf16, tag="wkc",
                                    name="wkc")
                wvc_sb = xpool.tile([128, NCT * C], bf16, tag="wvc",
                                    name="wvc")
                nc.sync.dma_start(wkc_sb[:], wkc)
                nc.sync.dma_start(wvc_sb[:], wvc)
                ctx_sb = xpool.tile([128, NCT * CTX], bf16, tag="ctx",
                                    name="ctx")
                nc.sync.dma_start(ctx_sb[:], ctx_bf)
                ctxa = lambda c: ctx_sb[:, c * CTX:(c + 1) * CTX]
                cqc0_sb = xpool.tile([1, C], bf16, tag="cqc", name="cqc0")
                nc.sync.dma_start(cqc0_sb[:], cqc0)

                # out-proj weights up front (all 6 live for wave 0)
                wot = [wstr.tile([128, NCT * 128], bf16, tag=f"wot{ot}",
                                 name=f"wot{ot}") for ot in range(NCT)]
                for ot in range(NCT):
                    nc.sync.dma_start(wot[ot][:],
                                      wo_t[:, ot * C:(ot + 1) * C])

                # ---- after each half-exchange: out-project that token
                # half; kc/vc projections fill the second A2A's wait ----
                oa = [xpool.tile([128, TG], bf16, tag=f"oa{k}",
                                 name=f"oa{k}") for k in range(NCT)]
                x2bf = [xpool.tile([128, TG], bf16, tag=f"x2bf{c}",
                                   name=f"x2bf{c}") for c in range(NCT)]
                s1 = ps_at2.tile([128, TG], f32, tag="s1", name="s1")
                s2 = ps_at2.tile([128, TG], f32, tag="s2", name="s2")
                for wv in range(2):
                    cs = slice(wv * 256, wv * 256 + 256)
                    for k in range(NCT):
                        nc.sync.dma_start(oa[k][:, cs],
                                          a2a_out[wv][k * 128:(k + 1) * 128,
                                                      :])
                    for ot in range(NCT):
                        ps = ps_pD.tile([128, 256], f32, tag="proj",
                                        name="ps")
                        for k in range(NCT):
                            nc.tensor.matmul(ps[:],
                                             wot[ot][:, k * 128:
                                                     (k + 1) * 128],
                                             oa[k][:, cs], start=(k == 0),
                                             stop=(k == NCT - 1))
                        nc.vector.tensor_tensor(x2[ot][:, cs], ps[:],
                                                xo_sb[ot][:, cs], alu.add)
                        nc.scalar.copy(x2bf[ot][:, cs], x2[ot][:, cs])
                        xsq = wpool.tile([128, 256], bf16,
                                         tag=f"xsq{ot % 2}", name="xsq",
                                         bufs=1)
                        nc.scalar.square(xsq[:], x2bf[ot][:, cs])
                        nc.tensor.matmul(s1[:, cs], ones_sb[:],
                                         x2bf[ot][:, cs],
                                         start=(ot == 0),
                                         stop=(ot == NCT - 1))
                        nc.tensor.matmul(s2[:, cs], ones_sb[:], xsq[:],
                                         start=(ot == 0),
                                         stop=(ot == NCT - 1))
                    if wv == 0:
                        # context-side projections (independent of A2A#2)
                        kc_sb = [xpool.tile([DH, CTX], bf16, tag=f"kc{h}",
                                            name=f"kc{h}") for h in range(H)]
                        for h in range(H):
                            ps = ps_pD.tile([DH, CTX], f32, tag="proj",
                                            name="ps")
                            for c in range(NCT):
                                nc.tensor.matmul(
                                    ps[:],
                                    wkc_sb[:, c * C + h * DH:
                                           c * C + (h + 1) * DH],
                                    ctxa(c), start=(c == 0),
                                    stop=(c == NCT - 1))
                            nc.vector.tensor_copy(kc_sb[h][:], ps[:])
                        vc_sb = xpool.tile([128, H * (DH + 1)], bf16,
                                           tag="vc", name="vc")
                        for half in range(2):
                            ps = ps_pD.tile([128, C // 2], f32, tag="proj",
                                            name="ps")
                            for c in range(NCT):
                                nc.tensor.matmul(
                                    ps[:], ctxa(c),
                                    wvc_sb[:, c * C + half * 384:
                                           c * C + (half + 1) * 384],
                                    start=(c == 0), stop=(c == NCT - 1))
                            dv = vc_sb[:].rearrange(
                                "p (h d) -> p h d", h=H)[
                                :, half * 4:(half + 1) * 4, 0:DH]
                            sv = ps[:].rearrange("p (h d) -> p h d", h=4)
                            nc.vector.tensor_copy(dv, sv)
                        nc.vector.memset(
                            vc_sb[:].rearrange("p (h d) -> p h d",
                                               h=H)[:, :, DH:DH + 1], 1.0)

                # ---- LN2 + cross-attention.  b1 is factored out of the
                # qc projection (applied per-column after the matmul), so
                # the rsqrt chain overlaps the qc matmuls.  Normalization
                # of the 8 heads' outputs is batched: one [8,TG]
                # reciprocal + 6 one-hot broadcast matmuls. ----
                b1, _, _ = ln_finish(s1, s2, TG, False, want_corr=False)
                mu_s1 = spool.tile([1, TG], bf16, tag="mu_s1", name="mu_s1")
                nc.vector.tensor_copy(mu_s1[:], s1[0:1, :])
                ocfm = [xpool.tile([128, TG], bf16, tag=f"oa{c}",
                                   name=f"ocfm{c}") for c in range(NCT)]
                wqct = [wstr.tile([128, NCT * DH], bf16, tag="wqct",
                                  name="wqct", bufs=4) for h in range(H)]
                for h in range(H):
                    nc.sync.dma_start(
                        wqct[h][:],
                        wqc_t[:, h * NCT * DH:(h + 1) * NCT * DH])
                with nc.allow_low_precision(reason="softmax reciprocal bf16"):
                    for h in range(H):
                        qc_ps = ps_pD.tile([DH, TG], f32, tag="proj",
                                           name="ps")
                        for c in range(NCT):
                            nc.tensor.matmul(
                                qc_ps[:], wqct[h][:, c * DH:(c + 1) * DH],
                                x2bf[c][:], start=(c == 0), stop=False)
                        nc.tensor.matmul(qc_ps[:],
                                         cqc0_sb[:, h * DH:(h + 1) * DH],
                                         mu_s1[:], start=False, stop=True)
                        qc = wpool.tile([DH, TG], bf16, tag="qc",
                                        name="qc", bufs=3)
                        nc.vector.tensor_tensor(qc[:], qc_ps[:],
                                                b1[0:DH, :], alu.mult)
                        s_ps = ps_at2.tile([CTX, TG], f32, tag="scores",
                                           name="s_ps", bufs=2)
                        nc.tensor.matmul(s_ps[:], kc_sb[h][:], qc[:],
                                         start=True, stop=True)
                        pt = wpool.tile([CTX, TG], bf16, tag="ptD",
                                        name="pt", bufs=3)
                        nc.scalar.activation(pt[:], s_ps[:], AF.Exp)
                        o_ps = ps_at2.tile([DH + 1, TG], f32, tag="av",
                                           name="o_ps", bufs=2)
                        nc.tensor.matmul(
                            o_ps[:],
                            vc_sb[:, h * (DH + 1):(h + 1) * (DH + 1)],
                            pt[:], start=True, stop=True)
                        o_raw = wpool.tile([DH, TG], bf16, tag="orawD",
                                           name="o_raw", bufs=2)
                        nc.scalar.copy(o_raw[:], o_ps[0:DH, :])
                        den = spool.tile([1, TG], f32, tag="denD",
                                         name="den", bufs=2)
                        nc.vector.tensor_copy(den[:], o_ps[DH:DH + 1, :])
                        rcp = spool.tile([1, TG], f32, tag="rcpD",
                                         name="rcp", bufs=2)
                        nc.vector.reciprocal_approx_fast(rcp[:], den[:])
                        rcpb1 = spool.tile([1, TG], bf16, tag="rcpbD",
                                           name="rcpb1", bufs=2)
                        nc.vector.tensor_copy(rcpb1[:], rcp[:])
                        b_ps = ps_at2.tile([128, TG], f32, tag="av",
                                           name="b_ps", bufs=2)
                        nc.tensor.matmul(b_ps[:], ones_sb[0:1, :],
                                         rcpb1[:], start=True, stop=True)

                        def _maxn(v):
                            if v % 128 == 0:
                                return 128
                            if v % 64 == 0:
                                return 64
                            return 32
                        pos = 0
                        while pos < DH:
                            r = h * DH + pos
                            c0, off = r // 128, r % 128
                            n = min(_maxn(off), _maxn(pos), DH - pos,
                                    128 - off)
                            nc.vector.tensor_tensor(
                                ocfm[c0][off:off + n, :],
                                b_ps[pos:pos + n, :],
                                o_raw[pos:pos + n, :], alu.mult)
                            pos += n

                # cross out-proj + residual; LN3 stats accumulate per tile
                x3bf = [xpool.tile([128, TG], bf16, tag=f"oa{c}",
                                   name=f"x3bf{c}") for c in range(NCT)]
                s1b = ps_at2.tile([128, TG], f32, tag="s1", name="s1b")
                s2b = ps_at2.tile([128, TG], f32, tag="s2", name="s2b")
                for ot in range(NCT):
                    wcot = wstr.tile([128, NCT * 128], bf16, tag="wcot",
                                     name="wcot", bufs=3)
                    nc.sync.dma_start(wcot[:],
                                      wco_t[:, ot * C:(ot + 1) * C])
                    ps = ps_pD.tile([128, TG], f32, tag="proj", name="ps")
                    for c in range(NCT):
                        nc.tensor.matmul(ps[:],
                                         wcot[:, c * 128:(c + 1) * 128],
                                         ocfm[c][:], start=(c == 0),
                                         stop=(c == NCT - 1))
                    nc.vector.tensor_tensor(x3[ot][:], ps[:], x2[ot][:],
                                            alu.add)
                    nc.scalar.copy(x3bf[ot][:], x3[ot][:])
                    xsq = wpool.tile([128, TG], bf16, tag=f"xsq{ot % 2}",
                                     name="xsq", bufs=1)
                    nc.scalar.square(xsq[:], x3bf[ot][:])
                    nc.tensor.matmul(s1b[:], ones_sb[:], x3bf[ot][:],
                                     start=(ot == 0), stop=(ot == NCT - 1))
                    nc.tensor.matmul(s2b[:], ones_sb[:], xsq[:],
                                     start=(ot == 0), stop=(ot == NCT - 1))

                # ---- LN3 (mean-subtracted on DVE, fp8 DoubleRow pack) ----
                b1, corr, mu_bc = ln_finish(s1b, s2b, TG, True, b1tag="b1c",
                                            want_corr=False)
                z3p = [xpool.tile([128, 2 * TG], fp8, tag=f"z3p{cp}",
                                  name=f"z3p{cp}") for cp in range(3)]
                for c in range(NCT):
                    zt = wpool.tile([128, TG], bf16, tag="zt", name="zt")
                    nc.vector.tensor_tensor(zt[:], x3bf[c][:], mu_bc[:],
                                            alu.subtract)
                    dst = z3p[c // 2][:, (c % 2) * TG:(c % 2 + 1) * TG]
                    nc.vector.tensor_tensor(dst, zt[:], b1[:], alu.mult)
                if not bias_zero:
                    onerow = xpool.tile([1, TG], bf16, tag="onerow",
                                        name="onerow")
                    nc.vector.memset(onerow[:], 1.0)
                    cg_sb = xpool.tile([1, II], bf16, tag="cg", name="cg")
                    cu_sb = xpool.tile([1, II], bf16, tag="cu", name="cu")
                    nc.sync.dma_start(cg_sb[:], cg)
                    nc.sync.dma_start(cu_sb[:], cu)

                # ---- Phase E: SwiGLU FFN, fp8 DoubleRow (weights x64) ----
                hh = [xpool.tile([128, 2 * TG], fp8, tag=f"hh{ip}",
                                 name=f"hh{ip}") for ip in range(12)]
                DR = mybir.MatmulPerfMode.DoubleRow
                for it in range(NIT):
                    wgu = wstr.tile([128, 2 * 768], fp8, tag="wgu",
                                    name="wgu", bufs=3)
                    nc.sync.dma_start(
                        wgu[:], wgu_t[:, it * 1536:(it + 1) * 1536])
                    g_ps = ps_at2.tile([128, TG], f32, tag="scores",
                                       name="g_ps", bufs=2)
                    u_ps = ps_pD.tile([128, TG], f32, tag="proj",
                                      name="u_ps")
                    for cp in range(3):
                        last = (cp == 2) and bias_zero
                        zr = z3p[cp][:].rearrange("p (r t) -> p r t", r=2)
                        for wofs, ps_ in ((0, g_ps), (768, u_ps)):
                            wr = wgu[:, wofs + cp * 256:
                                     wofs + (cp + 1) * 256].rearrange(
                                "p (r m) -> p r m", r=2)
                            nc.tensor.matmul(ps_[:], wr, zr,
                                             start=(cp == 0), stop=last,
                                             perf_mode=DR)
                    if not bias_zero:
                        nc.tensor.matmul(
                            g_ps[:], cg_sb[:, it * 128:(it + 1) * 128],
                            onerow[:], start=False, stop=True)
                        nc.tensor.matmul(
                            u_ps[:], cu_sb[:, it * 128:(it + 1) * 128],
                            onerow[:], start=False, stop=True)
                    # silu(g_true)*u_true: ACT Silu descales g (x1/64),
                    # the u descale (1/64) folds into the hh write
                    sg = wpool.tile([128, TG], bf16, tag="sg", name="sg")
                    nc.scalar.activation(sg[:], g_ps[:], AF.Silu,
                                         scale=1.0 / 64)
                    hdst = hh[it // 2][:, (it % 2) * TG:(it % 2 + 1) * TG]
                    nc.vector.scalar_tensor_tensor(hdst, sg[:], 1.0 / 64,
                                                   u_ps[:], alu.mult,
                                                   alu.mult)
                for ot in range(NCT):
                    wdt = wstr.tile([128, 12 * 256], fp8, tag="wdt",
                                    name="wdt", bufs=2)
                    nc.sync.dma_start(
                        wdt[:], wd_t[:, ot * 12 * 256:(ot + 1) * 12 * 256])
                    d_ps = ps_at2.tile([128, TG], f32, tag="scores",
                                       name="d_ps", bufs=2)
                    for ip in range(12):
                        wr = wdt[:, ip * 256:(ip + 1) * 256].rearrange(
                            "p (r m) -> p r m", r=2)
                        hr = hh[ip][:].rearrange("p (r t) -> p r t", r=2)
                        nc.tensor.matmul(d_ps[:], wr, hr,
                                         start=(ip == 0), stop=(ip == 11),
                                         perf_mode=DR)
                    xf = wpool.tile([128, TG], f32, tag="xf", name="xf")
                    nc.vector.scalar_tensor_tensor(xf[:], d_ps[:],
                                                   1.0 / 64, x3[ot][:],
                                                   alu.mult, alu.add)
                    nc.sync.dma_start(out_x[ot * 128:(ot + 1) * 128, :],
                                      xf[:])

    nc.compile()
    return nc


def _rope_tables(head_dim, height, width, frames, base=10000.0):
    d = head_dim // 3
    dx, dy, dt_ = d, d, head_dim - 2 * d

    def freqs(n, dd):
        inv = 1.0 / base ** (np.arange(0, dd, 2, dtype=np.float32) / dd)
        f = np.outer(np.arange(n, dtype=np.float32), inv)
        return np.concatenate([f, f], axis=-1)

    fx, fy, ft = freqs(width, dx), freqs(height, dy), freqs(frames, dt_)
    shp = (frames, height, width)
    cx = np.broadcast_to(np.cos(fx)[None, None, :, :], shp + (dx,))
    sx = np.broadcast_to(np.sin(fx)[None, None, :, :], shp + (dx,))
    cy = np.broadcast_to(np.cos(fy)[None, :, None, :], shp + (dy,))
    sy = np.broadcast_to(np.sin(fy)[None, :, None, :], shp + (dy,))
    ct = np.broadcast_to(np.cos(ft)[:, None, None, :], shp + (dt_,))
    st = np.broadcast_to(np.sin(ft)[:, None, None, :], shp + (dt_,))
    cos = np.concatenate([cx, cy, ct], axis=-1).reshape(-1, head_dim)
    sin = np.concatenate([sx, sy, st], axis=-1).reshape(-1, head_dim)
    return cos.astype(np.float32), sin.astype(np.float32)


def _qk_perm():
    """Stored-index -> original head-dim map (-1 = zero pad), length 128.
    Layout [x1(48) pad16 | x2(48) pad16] puts every rotate-half partner at
    stored index s^64."""
    P = np.full(DP, -1, np.int64)
    P[0:48] = np.arange(0, 48)
    P[64:112] = np.arange(48, 96)
    return P


def _tile6(W, nb):
    """[C, nb*128] -> [128, nb*NCT*128] with block (b, c) at cols
    (b*NCT+c)*128."""
    return np.ascontiguousarray(
        W.reshape(NCT, 128, nb, 128).transpose(1, 2, 0, 3).reshape(
            128, nb * NCT * 128))


def _prep_inputs(inputs):
    """Host-side prep.  Returns (bias_zero, in_maps)."""
    f = lambda k: np.asarray(inputs[k], np.float32)
    x, context = f("x"), f("context")
    wqkv, w_attn_out = f("wqkv"), f("w_attn_out")
    ln1_g, ln1_b = f("ln1_g"), f("ln1_b")
    wq_c, wk_c, wv_c, w_cross_out = (f("wq_c"), f("wk_c"), f("wv_c"),
                                     f("w_cross_out"))
    ln2_g, ln2_b = f("ln2_g"), f("ln2_b")
    w_gate, w_up, w_down = f("w_gate"), f("w_up"), f("w_down")
    ln3_g, ln3_b = f("ln3_g"), f("ln3_b")
    height, width, frames = (int(inputs["height"]), int(inputs["width"]),
                             int(inputs["frames"]))

    bias_zero = bool((ln3_b == 0).all())
    sc = DH ** -0.25

    # LN1 applied host-side (gain+bias folded into x-hat)
    mu = x.mean(axis=-1, keepdims=True)
    var = x.var(axis=-1, keepdims=True)
    xhat = (x - mu) / np.sqrt(var + EPS) * ln1_g + ln1_b

    def fold(W, g, b, scale=1.0):
        Wg = g[:, None] * W * scale
        c0 = -Wg.sum(axis=0)
        c1 = b @ W * scale
        return Wg, np.stack([c0, c1]).astype(BF16)

    assert (ln2_b == 0).all(), "factored-qc path assumes ln2_b == 0"
    wqc_g, cqc = fold(wq_c, ln2_g, ln2_b, sc)
    cqc0r = (cqc[0:1].astype(np.float32) / C).astype(BF16)   # -colsum/C
    wkc_s = (wk_c * sc).astype(BF16)
    # LN3: mean handled on-chip; fold only the gain.
    wg_g = (ln3_g[:, None] * w_gate).astype(BF16)
    wu_g = (ln3_g[:, None] * w_up).astype(BF16)

    cos, sin = _rope_tables(DH, height, width, frames)
    sinp = sin.copy()
    sinp[:, :DH // 2] *= -1.0
    P = _qk_perm()
    valid = P >= 0
    Pc = np.where(valid, P, 0)
    cosP = np.where(valid[None, :], cos[:, Pc], 0.0)
    sinP = np.where(valid[None, :], sinp[:, Pc], 0.0)
    cosT = np.ascontiguousarray(cosP.T).astype(BF16)
    # sin is read at raw's partition base (SB inputs must share it), so
    # pre-swap columns: sin_sb[d] = sinP[d^64], giving
    # out[d] = raw[d]*cosP[d] + raw[d^64]*sin_sb[d^64] = ... + raw[d^64]*sinP[d]
    sinT = np.ascontiguousarray(sinP[:, np.arange(DP) ^ 64].T).astype(BF16)

    def permute_qk(Wh):  # [rows, DH] -> [rows, DP] permuted+padded
        out = np.zeros((Wh.shape[0], DP), Wh.dtype)
        out[:, valid] = Wh[:, Pc[valid]]
        return out

    tri = np.triu(np.ones((128, 128), np.float32)).astype(BF16)
    ones128 = np.ones((128, 128), np.float32).astype(BF16)

    xT = np.ascontiguousarray(x.transpose(0, 2, 1))          # [B, C, S]
    xhT = np.ascontiguousarray(xhat.transpose(0, 2, 1))      # [B, C, S]
    ctxT = np.ascontiguousarray(context.transpose(0, 2, 1))  # [B, C, CTX]

    # pre-tiled streamed weights (shared across cores)
    wqc_tl = np.ascontiguousarray(
        wqc_g.astype(BF16).reshape(NCT, 128, H, DH).transpose(
            1, 2, 0, 3).reshape(128, H * NCT * DH))
    wo_tl = _tile6(w_attn_out.astype(BF16), NCT)
    wco_tl = _tile6(w_cross_out.astype(BF16), NCT)
    FP8 = ml_dtypes.float8_e4m3fn

    def pack_dr(W, nb):
        # [K, nb*128] -> [128, nb*(K/256)*256] fp8 DoubleRow blocks.
        # Slot (p, parity r) holds contraction row kp*256 + r*128 + p,
        # matching how the kernel packs z3/hh pairs on-chip:
        # lhsT[p, ((b*KP + kp)*128 + m)*2 + r] = W[kp*256 + r*128 + p, b*128+m]
        K = W.shape[0]
        KP = K // 256
        t = W.reshape(KP, 2, 128, nb, 128)          # [kp, r, p, b, m]
        t = t.transpose(2, 3, 0, 1, 4)              # [p, b, kp, r, m]
        return np.ascontiguousarray(t.reshape(128, nb * KP * 256)).astype(FP8)

    wg_tl = pack_dr(np.float32(64.0) * wg_g.astype(np.float32), NIT)
    wu_tl = pack_dr(np.float32(64.0) * wu_g.astype(np.float32), NIT)
    # merge gate/up blocks: [wg block it (768) | wu block it (768)]
    wgu_tl = np.empty((128, NIT * 1536), FP8)
    for it in range(NIT):
        wgu_tl[:, it * 1536:it * 1536 + 768] = \
            wg_tl[:, it * 768:(it + 1) * 768]
        wgu_tl[:, it * 1536 + 768:(it + 1) * 1536] = \
            wu_tl[:, it * 768:(it + 1) * 768]
    wd_tl = pack_dr(np.float32(64.0) * w_down, NCT)

    def xtile(xb):  # [C, S] -> [128, NMG*NCT*MG], block (g, c)
        return np.ascontiguousarray(
            xb.reshape(NCT, 128, NMG, MG).transpose(1, 2, 0, 3).reshape(
                128, NMG * NCT * MG))

    def rowtile(W, w):  # [C, w] -> [128, NCT*w], block c at cols c*w
        return np.ascontiguousarray(
            W.reshape(NCT, 128, w).transpose(1, 0, 2).reshape(128, NCT * w))

    shared = dict(
        cosT=cosT, sinT=sinT, tri=tri, ones_in=ones128,
        xh0=xtile(xhT[0].astype(BF16)), xh1=xtile(xhT[1].astype(BF16)),
        wo_t=wo_tl, wqc_t=wqc_tl, wco_t=wco_tl,
        wkc=rowtile(wkc_s, C), wvc=rowtile(wv_c.astype(BF16), C),
        wgu_t=np.ascontiguousarray(wgu_tl), wd_t=wd_tl,
        cqc0=cqc0r,
    )
    if not bias_zero:
        shared["cg"] = (ln3_b @ w_gate).astype(BF16)[None, :]
        shared["cu"] = (ln3_b @ w_up).astype(BF16)[None, :]
    in_maps = []
    for core in range(NCORES):
        h = core                      # head owned in phases A/B
        b, gq = core // 4, core % 4   # batch/token-group in phases D/E
        m = dict(shared)
        tok = np.r_[gq * 256:(gq + 1) * 256,
                    1024 + gq * 256:1024 + (gq + 1) * 256]
        m["x_own"] = np.ascontiguousarray(xT[b][:, tok])
        m["ctx_bf"] = rowtile(ctxT[b].astype(BF16), CTX)
        qs = slice(DH * h, DH * (h + 1))
        wq_h = permute_qk(wqkv[:, :C][:, qs] * sc).astype(BF16)
        wk_h = permute_qk(wqkv[:, C:2 * C][:, qs] * sc).astype(BF16)
        wv_h = wqkv[:, 2 * C:][:, qs].astype(BF16)
        # merged [wq|wk|wv] per 128-row tile: [128, NCT*(2*DP+DH)]
        wqkv_h = np.concatenate([wq_h, wk_h, wv_h], axis=1)  # [C, 352]
        m["wqkv_t"] = np.ascontiguousarray(
            wqkv_h.reshape(NCT, 128, 2 * DP + DH).transpose(1, 0, 2).reshape(
                128, NCT * (2 * DP + DH)))
        in_maps.append(m)
    return bias_zero, in_maps


def _get_nc(inputs):
    bias_zero, in_maps = _prep_inputs(inputs)
    key = ("nc", bias_zero)
    if key not in _CACHE:
        _CACHE[key] = _build_program(bias_zero)
    return _CACHE[key], in_maps


def kernel(**inputs):
    from concourse import bass_utils
    nc, in_maps = _get_nc(inputs)
    res = bass_utils.run_bass_kernel_spmd(nc, in_maps,
                                          core_ids=list(range(NCORES)))
    out = np.empty((B, C, S), np.float32)
    for core in range(NCORES):
        b, g = core // 4, core % 4
        r = res.results[core]["out_x"]
        out[b][:, g * 256:(g + 1) * 256] = r[:, 0:256]
        out[b][:, 1024 + g * 256:1024 + (g + 1) * 256] = r[:, 256:512]
    return np.ascontiguousarray(out.transpose(0, 2, 1))
